# revision 17
# baseline (speedup 1.0000x reference)
"""Trainium2 Bass kernel for nn_NoConsolidationModel (scatter_memory).

Math: per batch element with window w = seqs[b, 55:63], query q:
    h   = relu(concat(embed[q], mean_j embed[w_j]) @ W1.T + b1)
    out = h @ W2.T + b2
Folding embed into layer 1 (linearity):
    E1a  = embed @ W1[:, :64].T          # [66, 64]  (query part)
    E1bm = (embed @ W1[:, 64:].T) / 8    # [66, 64]  (window part)
    h_pre = E1a[q] + E1bm.T @ counts(w) + b1

Key trick vs the one-hot formulation: the whole batch is sorted by q and
dealt round-robin to the 8 cores, so each core's columns are grouped by
query value with identical group offsets on every core (SPMD-safe).
Within a q-group the query contribution is a constant vector, and since
counts always sum to 8 it folds into the count weights exactly:
    w_g = E1bm + ((E1a[g] + b1) / 8) 1^T   =>   w_g.T c = h_pre
so the device input is just the count matrix: 66 fp8 bytes/element
(vs 128 for counts+one-hot), and L1 needs no extra rows or bias.

Device layout (per core, n' columns, P = n'/2 psum columns):
  - L1: element col j -> psum(col j, rows 0:64) for j < P ("top"),
        psum(col j-P, rows 64:128) otherwise ("bottom").  The two halves
        are independent 64-col PE tiles -> concurrent streams.
        Matmuls break at q-group boundaries (per-segment weights).
  - ACT: relu PSUM->SBUF f16 (no bias needed, folded).
  - L2: W2.T stacked twice [128, 64]; top tile (0,0) and bottom tile
        (64,64) occupy disjoint PE quadrants -> concurrent streams.
  - DVE (mostly): +b2 with the PSUM->SBUF f16 copy; some groups go to
        ACT (Identity+bias) to balance the two engines.
  - Output f16 [64, n']: column c holds logits of element col c; each
    store is 8KB/partition contiguous.  Stores ride the SWDGE (gpsimd)
    ring so the input (sync/HWDGE) ring and ACT stay free.
Host: sorts, builds counts via bincount, unsorts the output.
"""

import sys

sys.path.insert(0, "/opt/trn_rl_repo")

import numpy as np
import ml_dtypes

B = 524288
NCORES = 8
V = 66          # VOCAB_SIZE + 2
H = 64          # HIDDEN_DIM
SEQ = 64
MEM = 8
WIN_LO = SEQ - 1 - MEM
WIN_HI = SEQ - 1
TS = 512        # matmul slice width (one PSUM bank of f32)
GW = 1024       # psum group width (ph/pl tile columns)
RW = 4096       # output region width (psum cols per out tile / store)

F8 = ml_dtypes.float8_e4m3

_PROG_CACHE = {}


def _plan(q):
    """Global layout plan shared by all cores: group offsets + column map.

    Returns (ofs, nprime, order, col, core) where element order[i] goes to
    core[i] at column col[i]; q-group g occupies columns [ofs[g], ofs[g+1])
    on every core.
    """
    q = np.asarray(q).astype(np.int64, copy=False)
    order = np.argsort(q, kind="stable")
    gcnt = np.bincount(q, minlength=V)                 # [V]
    s = -(-gcnt // NCORES)                             # ceil per-core size
    n_real = int(s.sum())
    nprime = -(-n_real // GW) * GW
    s[V - 1] += nprime - n_real                        # pad tail into last group
    ofs = np.zeros(V + 1, dtype=np.int64)
    np.cumsum(s, out=ofs[1:])
    # rank of each sorted element within its group
    gstart = np.zeros(V, dtype=np.int64)
    np.cumsum(gcnt[:-1], out=gstart[1:])
    p = np.arange(B, dtype=np.int64) - gstart[q[order]]
    core = (p % NCORES).astype(np.int64)
    col = ofs[q[order]] + p // NCORES
    return ofs, int(nprime), order, col, core


def _segments(ofs, nprime):
    """Per (group-tile, half): list of (local_a, local_b, qgroup, col_a)."""
    P = nprime // 2
    segs = {0: [], 1: []}
    for half in (0, 1):
        base = half * P
        # breakpoints: group boundaries clipped to [base, base+P) plus 512 grid
        bks = set(range(0, P + 1, TS))
        for g in range(V + 1):
            o = int(ofs[g]) - base
            if 0 < o < P:
                bks.add(o)
        bks = sorted(bks)
        for a, b in zip(bks[:-1], bks[1:]):
            g = int(np.searchsorted(ofs, base + a, side="right")) - 1
            segs[half].append((a, b, g))
    return segs


BW = 1024       # psum block width (X tile columns, 2 banks)


def _build_program(nprime, segs):
    import concourse.tile as tile
    from concourse import bacc, mybir

    P = nprime // 2
    n_blocks = -(-P // BW)
    f16 = mybir.dt.float16
    f32 = mybir.dt.float32
    f8 = mybir.dt.float8e4
    u8 = mybir.dt.uint8
    Relu = mybir.ActivationFunctionType.Relu
    Ident = mybir.ActivationFunctionType.Identity

    nc = bacc.Bacc("TRN2", target_bir_lowering=False, debug=False,
                   num_devices=NCORES)

    cnt_d = nc.dram_tensor("cnt", [V, nprime], f8, kind="ExternalInput").ap()
    # per-q-group folded L1 weights, group-major: [66, 66*64] f16 as u8
    wblob_d = nc.dram_tensor("wblob", [V, V * H * 2], u8,
                             kind="ExternalInput").ap()
    # [128, 132]: W2.T twice (128 B) | b2 twice f32 (4 B)
    cb2_d = nc.dram_tensor("cb2", [128, 132], u8, kind="ExternalInput").ap()
    out_d = nc.dram_tensor("out", [H, nprime], f16, kind="ExternalOutput").ap()

    # blocks whose b2-add goes to ACT instead of DVE (engine balance)
    act_badd = {3, 9, 15, 21, 27}

    # per-block instruction emitters, software-pipelined below
    def seg_list(k0, kw):
        """Interleaved top/bottom L1 segment matmul args for block cols
        [k0, k0+kw): list of (rows, a, b, g, cbase)."""
        out = []
        for half in (0, 1):
            lst = []
            for a, b, g in segs[half]:
                if a >= k0 + kw or b <= k0:
                    continue
                lst.append((slice(half * H, half * H + H), a, b, g, half * P))
            out.append(lst)
        merged = []
        i = j = 0
        t, bt = out[0], out[1]
        while i < len(t) or j < len(bt):
            if i < len(t):
                merged.append(t[i]); i += 1
            if j < len(bt):
                merged.append(bt[j]); j += 1
        return merged

    with tile.TileContext(nc) as tc:
        with (
            tc.tile_pool(name="const", bufs=1) as cpool,
            tc.tile_pool(name="cntp", bufs=1) as cnt_pool,
            tc.tile_pool(name="hbuf", bufs=3) as h_pool,
            tc.tile_pool(name="obuf", bufs=2) as out_pool,
            tc.tile_pool(name="px", bufs=4, space="PSUM") as x_pool,
        ):
            # constants ride the otherwise-idle SWDGE (gpsimd) ring so they
            # don't delay the count loads on the sync ring
            cb2_t = cpool.tile([128, 132], u8)
            nc.gpsimd.dma_start(cb2_t[:], cb2_d[:])
            w2t2 = cb2_t[:, 0:128].bitcast(f16)         # [128, 64]
            b2_s = cb2_t[:, 128:132].bitcast(f32)       # [128, 1]
            wb_t = cpool.tile([V, V * H * 2], u8)
            nc.gpsimd.dma_start(wb_t[:], wblob_d[:])
            wb = wb_t.bitcast(f16)                      # [66, 66*64]

            # whole-core count matrix; top-half loads on the sync HWDGE ring,
            # bottom-half on the SWDGE ring -> parallel issue + transfer
            cnt_t = cnt_pool.tile([V, nprime], f8)
            ld = P // 8
            for i in range(8):
                for half in (0, 1):
                    o = half * P + i * ld
                    w = ld if i < 7 else P - 7 * ld
                    eng = nc.sync if half == 0 else nc.gpsimd
                    eng.dma_start(cnt_t[:, o:o + w], cnt_d[:, o:o + w])

            x_tiles = [None] * 4
            out_t = [None]

            def emit_l1(k):
                k0 = k * BW
                kw = min(BW, P - k0)
                x = x_pool.tile([128, BW], f32, tag="x")
                x_tiles[k % 4] = x
                for rows, a, b, g, cbase in seg_list(k0, kw):
                    nc.tensor.matmul(x[rows, a - k0:b - k0],
                                     wb[:, g * H:(g + 1) * H],
                                     cnt_t[:, cbase + a:cbase + b],
                                     start=True, stop=True)

            def emit_tail(k):
                k0 = k * BW
                kw = min(BW, P - k0)
                x = x_tiles[k % 4]
                h_t = h_pool.tile([128, BW], f16, tag="h")
                nc.scalar.activation(h_t[:, :kw], x[:, :kw], Relu)
                # L2 writes back into the same X tile (after ACT read: WAR dep)
                for a in range(0, kw, TS):
                    b = min(a + TS, kw)
                    nc.tensor.matmul(x[0:H, a:b], w2t2[0:H, :],
                                     h_t[0:H, a:b], start=True, stop=True)
                    nc.tensor.matmul(x[H:128, a:b], w2t2[H:128, :],
                                     h_t[H:128, a:b], start=True, stop=True)
                # +b2 with the PSUM -> SBUF f16 copy, into the region out tile
                if k % 4 == 0:
                    out_t[0] = out_pool.tile([128, RW], f16, tag="o",
                                             name=f"ot{k}")
                ocols = slice((k % 4) * BW, (k % 4) * BW + kw)
                if k in act_badd:
                    nc.scalar.activation(out_t[0][:, ocols], x[:, :kw],
                                         Ident, bias=b2_s)
                else:
                    nc.vector.tensor_scalar_add(out_t[0][:, ocols],
                                                x[:, :kw], b2_s)
                if k % 4 == 3 or k == n_blocks - 1:
                    r0 = (k // 4) * RW
                    rw = min(RW, P - r0)
                    nc.sync.dma_start(out_d[:, r0:r0 + rw],
                                      out_t[0][0:H, :rw])
                    nc.sync.dma_start(out_d[:, P + r0:P + r0 + rw],
                                      out_t[0][H:128, :rw])

            # HAM warm-up: ~4us of dummy matmuls into an X tile while the
            # first loads are still in flight (PE would idle anyway); gets the
            # PE clock to 2.4GHz before real work starts.  The scratch input
            # comes from a DVE memset so no DMA gates the burst.
            ws_t = cpool.tile([128, 256], f16)
            nc.vector.memset(ws_t[:], 0.0)
            xw = x_pool.tile([128, BW], f32, tag="x")
            for i in range(18):
                nc.tensor.matmul(xw[:, 0:256], ws_t[:, 0:128],
                                 ws_t[:, 0:256], start=True, stop=True)

            emit_l1(0)
            emit_l1(1)
            for k in range(n_blocks):
                if k + 2 < n_blocks:
                    emit_l1(k + 2)
                emit_tail(k)

    nc.compile()
    return nc


def _get_program(nprime, segs):
    key = (nprime, tuple(segs[0]), tuple(segs[1]))
    if key not in _PROG_CACHE:
        _PROG_CACHE[key] = _build_program(nprime, segs)
    return _PROG_CACHE[key]


def _host_prep(seqs, query_tok, embed, W1, b1, W2, b2):
    embed = np.asarray(embed, dtype=np.float32)
    W1 = np.asarray(W1, dtype=np.float32)
    W2 = np.asarray(W2, dtype=np.float32)
    b1 = np.asarray(b1, dtype=np.float32)
    b2 = np.asarray(b2, dtype=np.float32)
    q = np.asarray(query_tok).astype(np.int64, copy=False)
    win = np.ascontiguousarray(np.asarray(seqs)[:, WIN_LO:WIN_HI]).astype(
        np.int64, copy=False)                                  # [B, MEM]

    ofs, nprime, order, col, core = _plan(q)
    segs = _segments(ofs, nprime)

    e1a = embed @ W1[:, :H].T                                  # [V, H]
    e1bm = (embed @ W1[:, H:].T) / MEM                         # [V, H]
    # folded per-group weights: w_g = e1bm + ((e1a[g] + b1)/8) 1^T
    wg = e1bm[None, :, :] + ((e1a + b1[None, :]) / MEM)[:, None, :]  # [V,V,H]
    wblob = np.ascontiguousarray(
        wg.transpose(1, 0, 2).reshape(V, V * H).astype(np.float16)
    ).view(np.uint8)                                           # [66, 66*64*2]
    w2t2 = np.ascontiguousarray(
        np.concatenate([W2.T, W2.T], axis=0)).astype(np.float16)   # [128, 64]
    b2x2 = np.concatenate([b2, b2]).reshape(128, 1).astype(np.float32)
    cb2 = np.concatenate([w2t2.view(np.uint8), b2x2.view(np.uint8)],
                         axis=1)                               # [128, 132]

    win_s = win[order]                                         # sorted by q
    cols64 = col
    in_maps = []
    for c in range(NCORES):
        m = core == c
        flat = (win_s[m] * nprime + cols64[m][:, None]).ravel()
        cnt = np.bincount(flat, minlength=V * nprime).astype(np.uint8)
        in_maps.append({
            "cnt": cnt.reshape(V, nprime).astype(F8),
            "wblob": wblob, "cb2": cb2,
        })
    return in_maps, (ofs, nprime, segs, order, col, core)


def _assemble(results, aux):
    ofs, nprime, segs, order, col, core = aux
    out = np.empty((B, H), dtype=np.float32)
    for c in range(NCORES):
        m = core == c
        out[order[m]] = results[c]["out"].astype(np.float32).T[col[m]]
    return out


def kernel(seqs, query_tok, embed, W1, b1, W2, b2):
    from concourse.bass_utils import run_bass_kernel_spmd

    in_maps, aux = _host_prep(seqs, query_tok, embed, W1, b1, W2, b2)
    nc = _get_program(aux[1], aux[2])
    res = run_bass_kernel_spmd(nc, in_maps, core_ids=list(range(NCORES)))
    return _assemble(res.results, aux)


# revision 19
# speedup vs baseline: 1.2787x; 1.2787x over previous
"""Trainium2 Bass kernel for nn_NoConsolidationModel (scatter_memory).

Math: per batch element with window w = seqs[b, 55:63], query q:
    h   = relu(concat(embed[q], mean_j embed[w_j]) @ W1.T + b1)
    out = h @ W2.T + b2
Folding embed into layer 1 (linearity):
    E1a  = embed @ W1[:, :64].T          # [66, 64]  (query part)
    E1bm = (embed @ W1[:, 64:].T) / 8    # [66, 64]  (window part)
    h_pre = E1a[q] + E1bm.T @ counts(w) + b1

Key trick vs the one-hot formulation: the whole batch is sorted by q and
dealt round-robin to the 8 cores, so each core's columns are grouped by
query value with identical group offsets on every core (SPMD-safe).
Within a q-group the query contribution is a constant vector, and since
counts always sum to 8 it folds into the count weights exactly:
    w_g = E1bm + ((E1a[g] + b1) / 8) 1^T   =>   w_g.T c = h_pre
so the device input is just the count matrix: 66 fp8 bytes/element
(vs 128 for counts+one-hot), and L1 needs no extra rows or bias.

Device layout (per core, n' columns, P = n'/2 psum columns):
  - L1: element col j -> psum(col j, rows 0:64) for j < P ("top"),
        psum(col j-P, rows 64:128) otherwise ("bottom").  The two halves
        are independent 64-col PE tiles -> concurrent streams.
        Matmuls break at q-group boundaries (per-segment weights).
  - ACT: relu PSUM->SBUF f16 (no bias needed, folded).
  - L2: W2.T stacked twice [128, 64]; top tile (0,0) and bottom tile
        (64,64) occupy disjoint PE quadrants -> concurrent streams.
  - DVE (mostly): +b2 with the PSUM->SBUF f16 copy; some groups go to
        ACT (Identity+bias) to balance the two engines.
  - Output f16 [64, n']: column c holds logits of element col c; each
    store is 8KB/partition contiguous.  Stores ride the SWDGE (gpsimd)
    ring so the input (sync/HWDGE) ring and ACT stay free.
Host: sorts, builds counts via bincount, unsorts the output.
"""

import sys

sys.path.insert(0, "/opt/trn_rl_repo")

import numpy as np
import ml_dtypes

B = 524288
NCORES = 8
V = 66          # VOCAB_SIZE + 2
H = 64          # HIDDEN_DIM
SEQ = 64
MEM = 8
WIN_LO = SEQ - 1 - MEM
WIN_HI = SEQ - 1
TS = 512        # matmul slice width (one PSUM bank of f32)
GW = 1024       # psum group width (ph/pl tile columns)
RW = 4096       # output region width (psum cols per out tile / store)

F8 = ml_dtypes.float8_e4m3

_PROG_CACHE = {}


def _plan(q):
    """Global layout plan shared by all cores: group offsets + column map.

    Returns (ofs, nprime, order, col, core) where element order[i] goes to
    core[i] at column col[i]; q-group g occupies columns [ofs[g], ofs[g+1])
    on every core.
    """
    q = np.asarray(q).astype(np.int64, copy=False)
    order = np.argsort(q, kind="stable")
    gcnt = np.bincount(q, minlength=V)                 # [V]
    s = -(-gcnt // NCORES)                             # ceil per-core size
    n_real = int(s.sum())
    nprime = -(-n_real // GW) * GW
    s[V - 1] += nprime - n_real                        # pad tail into last group
    ofs = np.zeros(V + 1, dtype=np.int64)
    np.cumsum(s, out=ofs[1:])
    # rank of each sorted element within its group
    gstart = np.zeros(V, dtype=np.int64)
    np.cumsum(gcnt[:-1], out=gstart[1:])
    p = np.arange(B, dtype=np.int64) - gstart[q[order]]
    core = (p % NCORES).astype(np.int64)
    col = ofs[q[order]] + p // NCORES
    return ofs, int(nprime), order, col, core


def _segments(ofs, nprime):
    """Per (group-tile, half): list of (local_a, local_b, qgroup, col_a)."""
    P = nprime // 2
    segs = {0: [], 1: []}
    for half in (0, 1):
        base = half * P
        # breakpoints: group boundaries clipped to [base, base+P) plus 512 grid
        bks = set(range(0, P + 1, TS))
        for g in range(V + 1):
            o = int(ofs[g]) - base
            if 0 < o < P:
                bks.add(o)
        bks = sorted(bks)
        for a, b in zip(bks[:-1], bks[1:]):
            g = int(np.searchsorted(ofs, base + a, side="right")) - 1
            segs[half].append((a, b, g))
    return segs


BW = 1024       # psum block width (X tile columns, 2 banks)


def _build_program(nprime, segs):
    import concourse.tile as tile
    from concourse import bacc, mybir

    P = nprime // 2
    n_blocks = -(-P // BW)
    f16 = mybir.dt.float16
    f32 = mybir.dt.float32
    f8 = mybir.dt.float8e4
    u8 = mybir.dt.uint8
    Relu = mybir.ActivationFunctionType.Relu
    Ident = mybir.ActivationFunctionType.Identity

    nc = bacc.Bacc("TRN2", target_bir_lowering=False, debug=False,
                   num_devices=NCORES)

    cnt_d = nc.dram_tensor("cnt", [V, nprime], f8, kind="ExternalInput").ap()
    # per-q-group folded L1 weights, group-major: [66, 66*64] f16 as u8
    wblob_d = nc.dram_tensor("wblob", [V, V * H * 2], u8,
                             kind="ExternalInput").ap()
    # [128, 132]: W2.T twice (128 B) | b2 twice f32 (4 B)
    cb2_d = nc.dram_tensor("cb2", [128, 132], u8, kind="ExternalInput").ap()
    out_d = nc.dram_tensor("out", [H, nprime], f16, kind="ExternalOutput").ap()

    # blocks whose b2-add goes to ACT instead of DVE (engine balance)
    act_badd = {3, 9, 15, 21, 27}

    # per-block instruction emitters, software-pipelined below
    def seg_list(k0, kw):
        """Interleaved top/bottom L1 segment matmul args for block cols
        [k0, k0+kw): list of (rows, a, b, g, cbase)."""
        out = []
        for half in (0, 1):
            lst = []
            for a, b, g in segs[half]:
                if a >= k0 + kw or b <= k0:
                    continue
                lst.append((slice(half * H, half * H + H), a, b, g, half * P))
            out.append(lst)
        merged = []
        i = j = 0
        t, bt = out[0], out[1]
        while i < len(t) or j < len(bt):
            if i < len(t):
                merged.append(t[i]); i += 1
            if j < len(bt):
                merged.append(bt[j]); j += 1
        return merged

    with tile.TileContext(nc) as tc:
        with (
            tc.tile_pool(name="const", bufs=1) as cpool,
            tc.tile_pool(name="cntp", bufs=1) as cnt_pool,
            tc.tile_pool(name="hbuf", bufs=3) as h_pool,
            tc.tile_pool(name="obuf", bufs=2) as out_pool,
            tc.tile_pool(name="px", bufs=4, space="PSUM") as x_pool,
        ):
            # constants ride the scalar HWDGE ring (ACT is idle during the
            # preamble) so they don't delay the count loads on the sync ring
            cb2_t = cpool.tile([128, 132], u8)
            nc.scalar.dma_start(cb2_t[:], cb2_d[:])
            w2t2 = cb2_t[:, 0:128].bitcast(f16)         # [128, 64]
            b2_s = cb2_t[:, 128:132].bitcast(f32)       # [128, 1]
            wb_t = cpool.tile([V, V * H * 2], u8)
            nc.scalar.dma_start(wb_t[:], wblob_d[:])
            wb = wb_t.bitcast(f16)                      # [66, 66*64]

            # whole-core count matrix; 16 loads interleaving the two halves so
            # early blocks get both halves first
            cnt_t = cnt_pool.tile([V, nprime], f8)
            ld = P // 8
            for i in range(8):
                for half in (0, 1):
                    o = half * P + i * ld
                    w = ld if i < 7 else P - 7 * ld
                    nc.sync.dma_start(cnt_t[:, o:o + w], cnt_d[:, o:o + w])

            x_tiles = [None] * 4
            out_t = [None]

            def emit_l1(k):
                k0 = k * BW
                kw = min(BW, P - k0)
                x = x_pool.tile([128, BW], f32, tag="x")
                x_tiles[k % 4] = x
                for rows, a, b, g, cbase in seg_list(k0, kw):
                    nc.tensor.matmul(x[rows, a - k0:b - k0],
                                     wb[:, g * H:(g + 1) * H],
                                     cnt_t[:, cbase + a:cbase + b],
                                     start=True, stop=True)

            def emit_tail(k):
                k0 = k * BW
                kw = min(BW, P - k0)
                x = x_tiles[k % 4]
                h_t = h_pool.tile([128, BW], f16, tag="h")
                nc.scalar.activation(h_t[:, :kw], x[:, :kw], Relu)
                # L2 writes back into the same X tile (after ACT read: WAR dep)
                for a in range(0, kw, TS):
                    b = min(a + TS, kw)
                    nc.tensor.matmul(x[0:H, a:b], w2t2[0:H, :],
                                     h_t[0:H, a:b], start=True, stop=True)
                    nc.tensor.matmul(x[H:128, a:b], w2t2[H:128, :],
                                     h_t[H:128, a:b], start=True, stop=True)
                # +b2 with the PSUM -> SBUF f16 copy, into the region out tile
                if k % 4 == 0:
                    out_t[0] = out_pool.tile([128, RW], f16, tag="o",
                                             name=f"ot{k}")
                ocols = slice((k % 4) * BW, (k % 4) * BW + kw)
                if k in act_badd:
                    nc.scalar.activation(out_t[0][:, ocols], x[:, :kw],
                                         Ident, bias=b2_s)
                else:
                    nc.vector.tensor_scalar_add(out_t[0][:, ocols],
                                                x[:, :kw], b2_s)
                if k % 4 == 3 or k == n_blocks - 1:
                    r0 = (k // 4) * RW
                    rw = min(RW, P - r0)
                    nc.sync.dma_start(out_d[:, r0:r0 + rw],
                                      out_t[0][0:H, :rw])
                    nc.sync.dma_start(out_d[:, P + r0:P + r0 + rw],
                                      out_t[0][H:128, :rw])

            # HAM warm-up: ~4us of dummy matmuls into an X tile while the
            # first loads are still in flight (PE would idle anyway); gets the
            # PE clock to 2.4GHz before real work starts.  The scratch input
            # comes from a DVE memset so no DMA gates the burst.
            ws_t = cpool.tile([128, 256], f16)
            nc.vector.memset(ws_t[:], 0.0)
            xw = x_pool.tile([128, BW], f32, tag="x")
            for i in range(22):
                nc.tensor.matmul(xw[:, 0:256], ws_t[:, 0:128],
                                 ws_t[:, 0:256], start=True, stop=True)

            emit_l1(0)
            emit_l1(1)
            for k in range(n_blocks):
                if k + 2 < n_blocks:
                    emit_l1(k + 2)
                emit_tail(k)

    nc.compile()
    return nc


def _get_program(nprime, segs):
    key = (nprime, tuple(segs[0]), tuple(segs[1]))
    if key not in _PROG_CACHE:
        _PROG_CACHE[key] = _build_program(nprime, segs)
    return _PROG_CACHE[key]


def _host_prep(seqs, query_tok, embed, W1, b1, W2, b2):
    embed = np.asarray(embed, dtype=np.float32)
    W1 = np.asarray(W1, dtype=np.float32)
    W2 = np.asarray(W2, dtype=np.float32)
    b1 = np.asarray(b1, dtype=np.float32)
    b2 = np.asarray(b2, dtype=np.float32)
    q = np.asarray(query_tok).astype(np.int64, copy=False)
    win = np.ascontiguousarray(np.asarray(seqs)[:, WIN_LO:WIN_HI]).astype(
        np.int64, copy=False)                                  # [B, MEM]

    ofs, nprime, order, col, core = _plan(q)
    segs = _segments(ofs, nprime)

    e1a = embed @ W1[:, :H].T                                  # [V, H]
    e1bm = (embed @ W1[:, H:].T) / MEM                         # [V, H]
    # folded per-group weights: w_g = e1bm + ((e1a[g] + b1)/8) 1^T
    wg = e1bm[None, :, :] + ((e1a + b1[None, :]) / MEM)[:, None, :]  # [V,V,H]
    wblob = np.ascontiguousarray(
        wg.transpose(1, 0, 2).reshape(V, V * H).astype(np.float16)
    ).view(np.uint8)                                           # [66, 66*64*2]
    w2t2 = np.ascontiguousarray(
        np.concatenate([W2.T, W2.T], axis=0)).astype(np.float16)   # [128, 64]
    b2x2 = np.concatenate([b2, b2]).reshape(128, 1).astype(np.float32)
    cb2 = np.concatenate([w2t2.view(np.uint8), b2x2.view(np.uint8)],
                         axis=1)                               # [128, 132]

    win_s = win[order]                                         # sorted by q
    cols64 = col
    in_maps = []
    for c in range(NCORES):
        m = core == c
        flat = (win_s[m] * nprime + cols64[m][:, None]).ravel()
        cnt = np.bincount(flat, minlength=V * nprime).astype(np.uint8)
        in_maps.append({
            "cnt": cnt.reshape(V, nprime).astype(F8),
            "wblob": wblob, "cb2": cb2,
        })
    return in_maps, (ofs, nprime, segs, order, col, core)


def _assemble(results, aux):
    ofs, nprime, segs, order, col, core = aux
    out = np.empty((B, H), dtype=np.float32)
    for c in range(NCORES):
        m = core == c
        out[order[m]] = results[c]["out"].astype(np.float32).T[col[m]]
    return out


def kernel(seqs, query_tok, embed, W1, b1, W2, b2):
    from concourse.bass_utils import run_bass_kernel_spmd

    in_maps, aux = _host_prep(seqs, query_tok, embed, W1, b1, W2, b2)
    nc = _get_program(aux[1], aux[2])
    res = run_bass_kernel_spmd(nc, in_maps, core_ids=list(range(NCORES)))
    return _assemble(res.results, aux)


# revision 22
# speedup vs baseline: 1.3268x; 1.0376x over previous
"""Trainium2 Bass kernel for nn_NoConsolidationModel (scatter_memory).

Math: per batch element with window w = seqs[b, 55:63], query q:
    h   = relu(concat(embed[q], mean_j embed[w_j]) @ W1.T + b1)
    out = h @ W2.T + b2
Folding embed into layer 1 (linearity):
    E1a  = embed @ W1[:, :64].T          # [66, 64]  (query part)
    E1bm = (embed @ W1[:, 64:].T) / 8    # [66, 64]  (window part)
    h_pre = E1a[q] + E1bm.T @ counts(w) + b1

Key trick vs the one-hot formulation: the whole batch is sorted by q and
dealt round-robin to the 8 cores, so each core's columns are grouped by
query value with identical group offsets on every core (SPMD-safe).
Within a q-group the query contribution is a constant vector, and since
counts always sum to 8 it folds into the count weights exactly:
    w_g = E1bm + ((E1a[g] + b1) / 8) 1^T   =>   w_g.T c = h_pre
so the device input is just the count matrix: 66 fp8 bytes/element
(vs 128 for counts+one-hot), and L1 needs no extra rows or bias.

Device layout (per core, n' columns, P = n'/2 psum columns):
  - L1: element col j -> psum(col j, rows 0:64) for j < P ("top"),
        psum(col j-P, rows 64:128) otherwise ("bottom").  The two halves
        are independent 64-col PE tiles -> concurrent streams.
        Matmuls break at q-group boundaries (per-segment weights).
  - ACT: relu PSUM->SBUF f16 (no bias needed, folded).
  - L2: W2.T stacked twice [128, 64]; top tile (0,0) and bottom tile
        (64,64) occupy disjoint PE quadrants -> concurrent streams.
  - DVE (mostly): +b2 with the PSUM->SBUF f16 copy; some groups go to
        ACT (Identity+bias) to balance the two engines.
  - Output f16 [64, n']: column c holds logits of element col c; each
    store is 8KB/partition contiguous.  Stores ride the SWDGE (gpsimd)
    ring so the input (sync/HWDGE) ring and ACT stay free.
Host: sorts, builds counts via bincount, unsorts the output.
"""

import sys

sys.path.insert(0, "/opt/trn_rl_repo")

import numpy as np
import ml_dtypes

B = 524288
NCORES = 8
V = 66          # VOCAB_SIZE + 2
H = 64          # HIDDEN_DIM
SEQ = 64
MEM = 8
WIN_LO = SEQ - 1 - MEM
WIN_HI = SEQ - 1
TS = 512        # matmul slice width (one PSUM bank of f32)
GW = 1024       # psum group width (ph/pl tile columns)
RW = 4096       # output region width (psum cols per out tile / store)

F8 = ml_dtypes.float8_e4m3

_PROG_CACHE = {}


def _plan(q):
    """Global layout plan shared by all cores: group offsets + column map.

    Returns (ofs, nprime, order, col, core) where element order[i] goes to
    core[i] at column col[i]; q-group g occupies columns [ofs[g], ofs[g+1])
    on every core.
    """
    q = np.asarray(q).astype(np.int64, copy=False)
    order = np.argsort(q, kind="stable")
    gcnt = np.bincount(q, minlength=V)                 # [V]
    s = -(-gcnt // NCORES)                             # ceil per-core size
    n_real = int(s.sum())
    nprime = -(-n_real // GW) * GW
    s[V - 1] += nprime - n_real                        # pad tail into last group
    ofs = np.zeros(V + 1, dtype=np.int64)
    np.cumsum(s, out=ofs[1:])
    # rank of each sorted element within its group
    gstart = np.zeros(V, dtype=np.int64)
    np.cumsum(gcnt[:-1], out=gstart[1:])
    p = np.arange(B, dtype=np.int64) - gstart[q[order]]
    core = (p % NCORES).astype(np.int64)
    col = ofs[q[order]] + p // NCORES
    return ofs, int(nprime), order, col, core


def _segments(ofs, nprime):
    """Per (group-tile, half): list of (local_a, local_b, qgroup, col_a)."""
    P = nprime // 2
    segs = {0: [], 1: []}
    for half in (0, 1):
        base = half * P
        # breakpoints: group boundaries clipped to [base, base+P) plus 512 grid
        bks = set(range(0, P + 1, TS))
        for g in range(V + 1):
            o = int(ofs[g]) - base
            if 0 < o < P:
                bks.add(o)
        bks = sorted(bks)
        for a, b in zip(bks[:-1], bks[1:]):
            g = int(np.searchsorted(ofs, base + a, side="right")) - 1
            segs[half].append((a, b, g))
    return segs


BW = 1024       # psum block width (X tile columns, 2 banks)


def _build_program(nprime, segs):
    import concourse.tile as tile
    from concourse import bacc, mybir

    P = nprime // 2
    n_blocks = -(-P // BW)
    f16 = mybir.dt.float16
    f32 = mybir.dt.float32
    f8 = mybir.dt.float8e4
    u8 = mybir.dt.uint8
    Relu = mybir.ActivationFunctionType.Relu
    Ident = mybir.ActivationFunctionType.Identity

    nc = bacc.Bacc("TRN2", target_bir_lowering=False, debug=False,
                   num_devices=NCORES)

    cnt_d = nc.dram_tensor("cnt", [V, nprime], f8, kind="ExternalInput").ap()
    # per-q-group folded L1 weights, group-major: [66, 66*64] f16 as u8
    wblob_d = nc.dram_tensor("wblob", [V, V * H * 2], u8,
                             kind="ExternalInput").ap()
    # [128, 132]: W2.T twice (128 B) | b2 twice f32 (4 B)
    cb2_d = nc.dram_tensor("cb2", [128, 132], u8, kind="ExternalInput").ap()
    out_d = nc.dram_tensor("out", [H, nprime], f16, kind="ExternalOutput").ap()

    # blocks whose b2-add goes to ACT instead of DVE (engine balance)
    act_badd = {10, 21}

    # per-block instruction emitters, software-pipelined below
    def seg_list(k0, kw):
        """Interleaved top/bottom L1 segment matmul args for block cols
        [k0, k0+kw): list of (rows, a, b, g, cbase)."""
        out = []
        for half in (0, 1):
            lst = []
            for a, b, g in segs[half]:
                if a >= k0 + kw or b <= k0:
                    continue
                lst.append((slice(half * H, half * H + H), a, b, g, half * P))
            out.append(lst)
        merged = []
        i = j = 0
        t, bt = out[0], out[1]
        while i < len(t) or j < len(bt):
            if i < len(t):
                merged.append(t[i]); i += 1
            if j < len(bt):
                merged.append(bt[j]); j += 1
        return merged

    with tile.TileContext(nc) as tc:
        with (
            tc.tile_pool(name="const", bufs=1) as cpool,
            tc.tile_pool(name="cntp", bufs=1) as cnt_pool,
            tc.tile_pool(name="hbuf", bufs=3) as h_pool,
            tc.tile_pool(name="obuf", bufs=2) as out_pool,
            tc.tile_pool(name="px", bufs=4, space="PSUM") as x_pool,
        ):
            # constants ride the scalar HWDGE ring (ACT is idle during the
            # preamble) so they don't delay the count loads on the sync ring
            cb2_t = cpool.tile([128, 132], u8)
            nc.scalar.dma_start(cb2_t[:], cb2_d[:])
            w2t2 = cb2_t[:, 0:128].bitcast(f16)         # [128, 64]
            b2_s = cb2_t[:, 128:132].bitcast(f32)       # [128, 1]
            wb_t = cpool.tile([V, V * H * 2], u8)
            nc.scalar.dma_start(wb_t[:], wblob_d[:])
            wb = wb_t.bitcast(f16)                      # [66, 66*64]

            # whole-core count matrix; 16 loads interleaving the two halves so
            # early blocks get both halves first
            cnt_t = cnt_pool.tile([V, nprime], f8)
            ld = P // 8
            for i in range(8):
                for half in (0, 1):
                    o = half * P + i * ld
                    w = ld if i < 7 else P - 7 * ld
                    nc.sync.dma_start(cnt_t[:, o:o + w], cnt_d[:, o:o + w])

            x_tiles = [None] * 4
            out_t = [None]

            def emit_l1(k):
                k0 = k * BW
                kw = min(BW, P - k0)
                x = x_pool.tile([128, BW], f32, tag="x")
                x_tiles[k % 4] = x
                for rows, a, b, g, cbase in seg_list(k0, kw):
                    nc.tensor.matmul(x[rows, a - k0:b - k0],
                                     wb[:, g * H:(g + 1) * H],
                                     cnt_t[:, cbase + a:cbase + b],
                                     start=True, stop=True)

            def emit_tail(k):
                k0 = k * BW
                kw = min(BW, P - k0)
                x = x_tiles[k % 4]
                h_t = h_pool.tile([128, BW], f16, tag="h")
                nc.scalar.activation(h_t[:, :kw], x[:, :kw], Relu)
                # L2 writes back into the same X tile (after ACT read: WAR dep)
                for a in range(0, kw, TS):
                    b = min(a + TS, kw)
                    nc.tensor.matmul(x[0:H, a:b], w2t2[0:H, :],
                                     h_t[0:H, a:b], start=True, stop=True)
                    nc.tensor.matmul(x[H:128, a:b], w2t2[H:128, :],
                                     h_t[H:128, a:b], start=True, stop=True)
                # +b2 with the PSUM -> SBUF f16 copy, into the region out tile
                if k % 4 == 0:
                    out_t[0] = out_pool.tile([128, RW], f16, tag="o",
                                             name=f"ot{k}")
                ocols = slice((k % 4) * BW, (k % 4) * BW + kw)
                if k in act_badd:
                    nc.scalar.activation(out_t[0][:, ocols], x[:, :kw],
                                         Ident, bias=b2_s)
                else:
                    nc.vector.tensor_scalar_add(out_t[0][:, ocols],
                                                x[:, :kw], b2_s)
                if k % 4 == 3 or k == n_blocks - 1:
                    r0 = (k // 4) * RW
                    rw = min(RW, P - r0)
                    nc.sync.dma_start(out_d[:, r0:r0 + rw],
                                      out_t[0][0:H, :rw])
                    nc.sync.dma_start(out_d[:, P + r0:P + r0 + rw],
                                      out_t[0][H:128, :rw])

            # HAM warm-up: ~4us of dummy matmuls into an X tile while the
            # first loads are still in flight (PE would idle anyway); gets the
            # PE clock to 2.4GHz before real work starts.  The scratch input
            # comes from a DVE memset so no DMA gates the burst.
            # burst must span >= 2 HAM windows (6.8us) to cover a full
            # free-running 4096-cycle window at any phase alignment
            ws_t = cpool.tile([128, 256], f16)
            nc.vector.memset(ws_t[:], 0.0)
            xw = x_pool.tile([128, BW], f32, tag="x")
            for i in range(36):
                nc.tensor.matmul(xw[:, 0:256], ws_t[:, 0:128],
                                 ws_t[:, 0:256], start=True, stop=True)

            emit_l1(0)
            emit_l1(1)
            for k in range(n_blocks):
                if k + 2 < n_blocks:
                    emit_l1(k + 2)
                emit_tail(k)

    nc.compile()
    return nc


def _get_program(nprime, segs):
    key = (nprime, tuple(segs[0]), tuple(segs[1]))
    if key not in _PROG_CACHE:
        _PROG_CACHE[key] = _build_program(nprime, segs)
    return _PROG_CACHE[key]


def _host_prep(seqs, query_tok, embed, W1, b1, W2, b2):
    embed = np.asarray(embed, dtype=np.float32)
    W1 = np.asarray(W1, dtype=np.float32)
    W2 = np.asarray(W2, dtype=np.float32)
    b1 = np.asarray(b1, dtype=np.float32)
    b2 = np.asarray(b2, dtype=np.float32)
    q = np.asarray(query_tok).astype(np.int64, copy=False)
    win = np.ascontiguousarray(np.asarray(seqs)[:, WIN_LO:WIN_HI]).astype(
        np.int64, copy=False)                                  # [B, MEM]

    ofs, nprime, order, col, core = _plan(q)
    segs = _segments(ofs, nprime)

    e1a = embed @ W1[:, :H].T                                  # [V, H]
    e1bm = (embed @ W1[:, H:].T) / MEM                         # [V, H]
    # folded per-group weights: w_g = e1bm + ((e1a[g] + b1)/8) 1^T
    wg = e1bm[None, :, :] + ((e1a + b1[None, :]) / MEM)[:, None, :]  # [V,V,H]
    wblob = np.ascontiguousarray(
        wg.transpose(1, 0, 2).reshape(V, V * H).astype(np.float16)
    ).view(np.uint8)                                           # [66, 66*64*2]
    w2t2 = np.ascontiguousarray(
        np.concatenate([W2.T, W2.T], axis=0)).astype(np.float16)   # [128, 64]
    b2x2 = np.concatenate([b2, b2]).reshape(128, 1).astype(np.float32)
    cb2 = np.concatenate([w2t2.view(np.uint8), b2x2.view(np.uint8)],
                         axis=1)                               # [128, 132]

    win_s = win[order]                                         # sorted by q
    cols64 = col
    in_maps = []
    for c in range(NCORES):
        m = core == c
        flat = (win_s[m] * nprime + cols64[m][:, None]).ravel()
        cnt = np.bincount(flat, minlength=V * nprime).astype(np.uint8)
        in_maps.append({
            "cnt": cnt.reshape(V, nprime).astype(F8),
            "wblob": wblob, "cb2": cb2,
        })
    return in_maps, (ofs, nprime, segs, order, col, core)


def _assemble(results, aux):
    ofs, nprime, segs, order, col, core = aux
    out = np.empty((B, H), dtype=np.float32)
    for c in range(NCORES):
        m = core == c
        out[order[m]] = results[c]["out"].astype(np.float32).T[col[m]]
    return out


def kernel(seqs, query_tok, embed, W1, b1, W2, b2):
    from concourse.bass_utils import run_bass_kernel_spmd

    in_maps, aux = _host_prep(seqs, query_tok, embed, W1, b1, W2, b2)
    nc = _get_program(aux[1], aux[2])
    res = run_bass_kernel_spmd(nc, in_maps, core_ids=list(range(NCORES)))
    return _assemble(res.results, aux)


# revision 23
# speedup vs baseline: 1.5427x; 1.1627x over previous
"""Trainium2 Bass kernel for nn_NoConsolidationModel (scatter_memory).

Math: per batch element with window w = seqs[b, 55:63], query q:
    h   = relu(concat(embed[q], mean_j embed[w_j]) @ W1.T + b1)
    out = h @ W2.T + b2
Folding embed into layer 1 (linearity):
    E1a  = embed @ W1[:, :64].T          # [66, 64]  (query part)
    E1bm = (embed @ W1[:, 64:].T) / 8    # [66, 64]  (window part)
    h_pre = E1a[q] + E1bm.T @ counts(w) + b1

Key trick vs the one-hot formulation: the whole batch is sorted by q and
dealt round-robin to the 8 cores, so each core's columns are grouped by
query value with identical group offsets on every core (SPMD-safe).
Within a q-group the query contribution is a constant vector, and since
counts always sum to 8 it folds into the count weights exactly:
    w_g = E1bm + ((E1a[g] + b1) / 8) 1^T   =>   w_g.T c = h_pre
so the device input is just the count matrix: 66 fp8 bytes/element
(vs 128 for counts+one-hot), and L1 needs no extra rows or bias.

Device layout (per core, n' columns, P = n'/2 psum columns):
  - L1: element col j -> psum(col j, rows 0:64) for j < P ("top"),
        psum(col j-P, rows 64:128) otherwise ("bottom").  The two halves
        are independent 64-col PE tiles -> concurrent streams.
        Matmuls break at q-group boundaries (per-segment weights).
  - ACT: relu PSUM->SBUF f16 (no bias needed, folded).
  - L2: W2.T stacked twice [128, 64]; top tile (0,0) and bottom tile
        (64,64) occupy disjoint PE quadrants -> concurrent streams.
  - DVE (mostly): +b2 with the PSUM->SBUF f16 copy; some groups go to
        ACT (Identity+bias) to balance the two engines.
  - Output f16 [64, n']: column c holds logits of element col c; each
    store is 8KB/partition contiguous.  Stores ride the SWDGE (gpsimd)
    ring so the input (sync/HWDGE) ring and ACT stay free.
Host: sorts, builds counts via bincount, unsorts the output.
"""

import sys

sys.path.insert(0, "/opt/trn_rl_repo")

import numpy as np
import ml_dtypes

B = 524288
NCORES = 8
V = 66          # VOCAB_SIZE + 2
H = 64          # HIDDEN_DIM
SEQ = 64
MEM = 8
WIN_LO = SEQ - 1 - MEM
WIN_HI = SEQ - 1
TS = 512        # matmul slice width (one PSUM bank of f32)
GW = 1024       # psum group width (ph/pl tile columns)
RW = 4096       # output region width (psum cols per out tile / store)

F8 = ml_dtypes.float8_e4m3

_PROG_CACHE = {}


def _plan(q):
    """Global layout plan shared by all cores: group offsets + column map.

    Returns (ofs, nprime, order, col, core) where element order[i] goes to
    core[i] at column col[i]; q-group g occupies columns [ofs[g], ofs[g+1])
    on every core.
    """
    q = np.asarray(q).astype(np.int64, copy=False)
    order = np.argsort(q, kind="stable")
    gcnt = np.bincount(q, minlength=V)                 # [V]
    s = -(-gcnt // NCORES)                             # ceil per-core size
    n_real = int(s.sum())
    nprime = -(-n_real // GW) * GW
    s[V - 1] += nprime - n_real                        # pad tail into last group
    ofs = np.zeros(V + 1, dtype=np.int64)
    np.cumsum(s, out=ofs[1:])
    # rank of each sorted element within its group
    gstart = np.zeros(V, dtype=np.int64)
    np.cumsum(gcnt[:-1], out=gstart[1:])
    p = np.arange(B, dtype=np.int64) - gstart[q[order]]
    core = (p % NCORES).astype(np.int64)
    col = ofs[q[order]] + p // NCORES
    return ofs, int(nprime), order, col, core


def _segments(ofs, nprime):
    """Per (group-tile, half): list of (local_a, local_b, qgroup, col_a)."""
    P = nprime // 2
    segs = {0: [], 1: []}
    for half in (0, 1):
        base = half * P
        # breakpoints: group boundaries clipped to [base, base+P) plus 512 grid
        bks = set(range(0, P + 1, TS))
        for g in range(V + 1):
            o = int(ofs[g]) - base
            if 0 < o < P:
                bks.add(o)
        bks = sorted(bks)
        for a, b in zip(bks[:-1], bks[1:]):
            g = int(np.searchsorted(ofs, base + a, side="right")) - 1
            segs[half].append((a, b, g))
    return segs


BW = 1024       # psum block width (X tile columns, 2 banks)


def _build_program(nprime, segs):
    import concourse.tile as tile
    from concourse import bacc, mybir

    P = nprime // 2
    n_blocks = -(-P // BW)
    f16 = mybir.dt.float16
    f32 = mybir.dt.float32
    f8 = mybir.dt.float8e4
    u8 = mybir.dt.uint8
    Relu = mybir.ActivationFunctionType.Relu
    Ident = mybir.ActivationFunctionType.Identity

    nc = bacc.Bacc("TRN2", target_bir_lowering=False, debug=False,
                   num_devices=NCORES)

    cnt_d = nc.dram_tensor("cnt", [V, nprime], f8, kind="ExternalInput").ap()
    # per-q-group folded L1 weights, group-major: [66, 66*64] f16 as u8
    wblob_d = nc.dram_tensor("wblob", [V, V * H * 2], u8,
                             kind="ExternalInput").ap()
    # [128, 132]: W2.T twice (128 B) | b2 twice f32 (4 B)
    cb2_d = nc.dram_tensor("cb2", [128, 132], u8, kind="ExternalInput").ap()
    out_d = nc.dram_tensor("out", [H, nprime], f16, kind="ExternalOutput").ap()

    # blocks whose b2-add goes to ACT instead of DVE (engine balance)
    act_badd = {10, 21}

    # per-block instruction emitters, software-pipelined below
    def seg_list(k0, kw):
        """Interleaved top/bottom L1 segment matmul args for block cols
        [k0, k0+kw): list of (rows, a, b, g, cbase)."""
        out = []
        for half in (0, 1):
            lst = []
            for a, b, g in segs[half]:
                if a >= k0 + kw or b <= k0:
                    continue
                lst.append((slice(half * H, half * H + H), a, b, g, half * P))
            out.append(lst)
        merged = []
        i = j = 0
        t, bt = out[0], out[1]
        while i < len(t) or j < len(bt):
            if i < len(t):
                merged.append(t[i]); i += 1
            if j < len(bt):
                merged.append(bt[j]); j += 1
        return merged

    with tile.TileContext(nc) as tc:
        with (
            tc.tile_pool(name="const", bufs=1) as cpool,
            tc.tile_pool(name="cntp", bufs=1) as cnt_pool,
            tc.tile_pool(name="hbuf", bufs=3) as h_pool,
            tc.tile_pool(name="obuf", bufs=3) as out_pool,
            tc.tile_pool(name="px", bufs=4, space="PSUM") as x_pool,
        ):
            # constants ride the scalar HWDGE ring (ACT is idle during the
            # preamble) so they don't delay the count loads on the sync ring
            cb2_t = cpool.tile([128, 132], u8)
            nc.scalar.dma_start(cb2_t[:], cb2_d[:])
            w2t2 = cb2_t[:, 0:128].bitcast(f16)         # [128, 64]
            b2_s = cb2_t[:, 128:132].bitcast(f32)       # [128, 1]
            wb_t = cpool.tile([V, V * H * 2], u8)
            nc.scalar.dma_start(wb_t[:], wblob_d[:])
            wb = wb_t.bitcast(f16)                      # [66, 66*64]

            # whole-core count matrix; 16 loads interleaving the two halves so
            # early blocks get both halves first
            cnt_t = cnt_pool.tile([V, nprime], f8)
            ld = P // 8
            for i in range(8):
                for half in (0, 1):
                    o = half * P + i * ld
                    w = ld if i < 7 else P - 7 * ld
                    nc.sync.dma_start(cnt_t[:, o:o + w], cnt_d[:, o:o + w])

            x_tiles = [None] * 4
            out_t = [None]

            def emit_l1(k):
                k0 = k * BW
                kw = min(BW, P - k0)
                x = x_pool.tile([128, BW], f32, tag="x")
                x_tiles[k % 4] = x
                for rows, a, b, g, cbase in seg_list(k0, kw):
                    nc.tensor.matmul(x[rows, a - k0:b - k0],
                                     wb[:, g * H:(g + 1) * H],
                                     cnt_t[:, cbase + a:cbase + b],
                                     start=True, stop=True)

            def emit_tail(k):
                k0 = k * BW
                kw = min(BW, P - k0)
                x = x_tiles[k % 4]
                h_t = h_pool.tile([128, BW], f16, tag="h")
                nc.scalar.activation(h_t[:, :kw], x[:, :kw], Relu)
                # L2 writes back into the same X tile (after ACT read: WAR dep)
                for a in range(0, kw, TS):
                    b = min(a + TS, kw)
                    nc.tensor.matmul(x[0:H, a:b], w2t2[0:H, :],
                                     h_t[0:H, a:b], start=True, stop=True)
                    nc.tensor.matmul(x[H:128, a:b], w2t2[H:128, :],
                                     h_t[H:128, a:b], start=True, stop=True)
                # +b2 with the PSUM -> SBUF f16 copy, into the region out tile
                if k % 4 == 0:
                    out_t[0] = out_pool.tile([128, RW], f16, tag="o",
                                             name=f"ot{k}")
                ocols = slice((k % 4) * BW, (k % 4) * BW + kw)
                if k in act_badd:
                    nc.scalar.activation(out_t[0][:, ocols], x[:, :kw],
                                         Ident, bias=b2_s)
                else:
                    nc.vector.tensor_scalar_add(out_t[0][:, ocols],
                                                x[:, :kw], b2_s)
                if k % 4 == 3 or k == n_blocks - 1:
                    r0 = (k // 4) * RW
                    rw = min(RW, P - r0)
                    nc.sync.dma_start(out_d[:, r0:r0 + rw],
                                      out_t[0][0:H, :rw])
                    nc.sync.dma_start(out_d[:, P + r0:P + r0 + rw],
                                      out_t[0][H:128, :rw])

            # HAM warm-up: ~4us of dummy matmuls into an X tile while the
            # first loads are still in flight (PE would idle anyway); gets the
            # PE clock to 2.4GHz before real work starts.  The scratch input
            # comes from a DVE memset so no DMA gates the burst.
            # burst must span >= 2 HAM windows (6.8us) to cover a full
            # free-running 4096-cycle window at any phase alignment
            ws_t = cpool.tile([128, 256], f16)
            nc.vector.memset(ws_t[:], 0.0)
            xw = x_pool.tile([128, BW], f32, tag="x")
            for i in range(36):
                nc.tensor.matmul(xw[:, 0:256], ws_t[:, 0:128],
                                 ws_t[:, 0:256], start=True, stop=True)

            emit_l1(0)
            emit_l1(1)
            for k in range(n_blocks):
                if k + 2 < n_blocks:
                    emit_l1(k + 2)
                emit_tail(k)

    nc.compile()
    return nc


def _get_program(nprime, segs):
    key = (nprime, tuple(segs[0]), tuple(segs[1]))
    if key not in _PROG_CACHE:
        _PROG_CACHE[key] = _build_program(nprime, segs)
    return _PROG_CACHE[key]


def _host_prep(seqs, query_tok, embed, W1, b1, W2, b2):
    embed = np.asarray(embed, dtype=np.float32)
    W1 = np.asarray(W1, dtype=np.float32)
    W2 = np.asarray(W2, dtype=np.float32)
    b1 = np.asarray(b1, dtype=np.float32)
    b2 = np.asarray(b2, dtype=np.float32)
    q = np.asarray(query_tok).astype(np.int64, copy=False)
    win = np.ascontiguousarray(np.asarray(seqs)[:, WIN_LO:WIN_HI]).astype(
        np.int64, copy=False)                                  # [B, MEM]

    ofs, nprime, order, col, core = _plan(q)
    segs = _segments(ofs, nprime)

    e1a = embed @ W1[:, :H].T                                  # [V, H]
    e1bm = (embed @ W1[:, H:].T) / MEM                         # [V, H]
    # folded per-group weights: w_g = e1bm + ((e1a[g] + b1)/8) 1^T
    wg = e1bm[None, :, :] + ((e1a + b1[None, :]) / MEM)[:, None, :]  # [V,V,H]
    wblob = np.ascontiguousarray(
        wg.transpose(1, 0, 2).reshape(V, V * H).astype(np.float16)
    ).view(np.uint8)                                           # [66, 66*64*2]
    w2t2 = np.ascontiguousarray(
        np.concatenate([W2.T, W2.T], axis=0)).astype(np.float16)   # [128, 64]
    b2x2 = np.concatenate([b2, b2]).reshape(128, 1).astype(np.float32)
    cb2 = np.concatenate([w2t2.view(np.uint8), b2x2.view(np.uint8)],
                         axis=1)                               # [128, 132]

    win_s = win[order]                                         # sorted by q
    cols64 = col
    in_maps = []
    for c in range(NCORES):
        m = core == c
        flat = (win_s[m] * nprime + cols64[m][:, None]).ravel()
        cnt = np.bincount(flat, minlength=V * nprime).astype(np.uint8)
        in_maps.append({
            "cnt": cnt.reshape(V, nprime).astype(F8),
            "wblob": wblob, "cb2": cb2,
        })
    return in_maps, (ofs, nprime, segs, order, col, core)


def _assemble(results, aux):
    ofs, nprime, segs, order, col, core = aux
    out = np.empty((B, H), dtype=np.float32)
    for c in range(NCORES):
        m = core == c
        out[order[m]] = results[c]["out"].astype(np.float32).T[col[m]]
    return out


def kernel(seqs, query_tok, embed, W1, b1, W2, b2):
    from concourse.bass_utils import run_bass_kernel_spmd

    in_maps, aux = _host_prep(seqs, query_tok, embed, W1, b1, W2, b2)
    nc = _get_program(aux[1], aux[2])
    res = run_bass_kernel_spmd(nc, in_maps, core_ids=list(range(NCORES)))
    return _assemble(res.results, aux)
